# revision 14
# baseline (speedup 1.0000x reference)
"""Distributed Bass kernel: RMSNorm + multi-head attention + out-proj on 8 TRN2 cores.

Sharding: head x batch tensor parallel. Core c owns batch c//4 and heads
[4*(c%4), 4*(c%4)+4) for the full 2048-token sequence. Each core RMSNorms the
whole batch, projects Q/K/V for only its 4 heads (w_qkv column shard), runs
full attention for those heads, and computes a partial output projection
(w_out row shard). A single bf16 ReduceScatter per token-half sums the 4
partials of each batch group and scatters 512 rows back to each core - the
only collective in the kernel (the baseline's 8 serialized K/V AllGathers
cost ~330us on the collective cores).

Attention pipeline per (head, 128-query tile): q-major sim on the PE
(fp16, x8 scale folded into w_q), exact row-max via DVE+Pool psum reduces,
one ScalarE exp pass (bias = -rowmax), DMA-xbar transpose of the bf16 attn
tile into keys-major layout, then a full-PE AV matmul (lhsT = attnT tile,
128x128 stationary; moving operand = [v | ones], 65 columns) whose extra
ones-column yields the softmax denominator for free. Normalization happens
on the tiny [128, 64] AV output, not the [128, 2048] attn matrix.
"""

import sys

sys.path.insert(0, "/opt/trn_rl_repo")

import numpy as np
import ml_dtypes

import concourse.bass as bass
import concourse.mybir as mybir
import concourse.tile as tile
from concourse import bacc
from concourse.bass_utils import run_bass_kernel_spmd
from concourse.masks import make_identity

F32 = mybir.dt.float32
F16 = mybir.dt.float16
BF16 = mybir.dt.bfloat16
AF = mybir.ActivationFunctionType
ALU = mybir.AluOpType

B, N, D = 2, 2048, 1024
H, DH = 16, 64
EPS = 1e-5
NC_TOTAL = 8
HPC = 4                 # heads per core
GROUP = 4               # cores per batch (reduce-scatter group)
NT = N // 128           # 16 token tiles
QT = NT                 # query tiles
KC = NT                 # key chunks of 128
DC = D // 128           # 8 contraction chunks
WQKV_COLS = 3 * HPC * DH  # 768


def build_graph():
    nc = bacc.Bacc(name="attn8")
    x_d = nc.dram_tensor("x", [N, D], F16, kind="ExternalInput")
    w_d = nc.dram_tensor("w_qkv", [D, WQKV_COLS], F16, kind="ExternalInput")
    wout_d = nc.dram_tensor("w_out", [HPC * DH, D], BF16, kind="ExternalInput")
    outp_d = nc.dram_tensor("outp", [N, D], BF16, kind="Internal")
    rsout_d = nc.dram_tensor("rsout", [N // GROUP, D], BF16, kind="Internal")
    out_d = nc.dram_tensor("out", [N // GROUP, D], BF16,
                           kind="ExternalOutput")  # [512, 1024]

    rg = [list(range(GROUP)), list(range(GROUP, 2 * GROUP))]

    with tile.TileContext(nc) as tc:
        with (
            tc.tile_pool(name="const", bufs=1) as constp,
            tc.tile_pool(name="xload", bufs=4) as xp,
            tc.tile_pool(name="xnorm", bufs=NT) as xnp,
            tc.tile_pool(name="xnT", bufs=DC) as xntp,
            tc.tile_pool(name="wqkv", bufs=DC) as wp,
            tc.tile_pool(name="wout", bufs=2) as woutp,
            tc.tile_pool(name="kq", bufs=2) as kqp,
            tc.tile_pool(name="vx", bufs=HPC) as vxp,
            tc.tile_pool(name="stats", bufs=8) as statsp,
            tc.tile_pool(name="scr", bufs=2) as scrp,
            tc.tile_pool(name="attn", bufs=4) as attnp,
            tc.tile_pool(name="attnT", bufs=7) as attntp,
            tc.tile_pool(name="aout", bufs=2 * QT) as aoutp,
            tc.tile_pool(name="aoutT", bufs=2) as aouttp,
            tc.tile_pool(name="osb", bufs=3) as osbp,
            tc.tile_pool(name="ps_a", bufs=3, space="PSUM") as psa,
            tc.tile_pool(name="ps_b", bufs=2, space="PSUM") as psb,
        ):
            identf = constp.tile([128, 128], F16, name="identf")
            make_identity(nc, identf[:])
            identb = constp.tile([128, 128], BF16, name="identb")
            make_identity(nc, identb[:])
            epsb = constp.tile([128, 1], F32, name="epsb")
            nc.any.memset(epsb[:], EPS)

            # ---------------- DMA loads ----------------
            xt = []
            for t in range(NT):
                xl = xp.tile([128, D], F16, name=f"x{t}", tag="x")
                nc.sync.dma_start(xl[:], x_d[t * 128:(t + 1) * 128, :])
                xt.append(xl)
            w_sb = []
            for dc in range(DC):
                w = wp.tile([128, WQKV_COLS], F16, name=f"w{dc}", tag="w")
                nc.sync.dma_start(w[:], w_d[dc * 128:(dc + 1) * 128, :])
                w_sb.append(w)
            wout_sb = []
            for i in range(2):
                w = woutp.tile([128, D], BF16, name=f"wo{i}", tag="wo")
                nc.sync.dma_start(w[:], wout_d[i * 128:(i + 1) * 128, :])
                wout_sb.append(w)

            # ---------------- RMSNorm (per token tile) ----------------
            xn = []
            for t in range(NT):
                scr = scrp.tile([128, D], F16, name=f"scr{t}", tag="scr")
                ssq = statsp.tile([128, 1], F32, name=f"ssq{t}", tag="ssq")
                nc.scalar.activation(scr[:], xt[t][:], AF.Square,
                                     accum_out=ssq[:])
                std = statsp.tile([128, 1], F32, name=f"std{t}", tag="ssq")
                nc.scalar.activation(std[:], ssq[:], AF.Sqrt, scale=1.0 / D,
                                     bias=epsb[:])
                rinv = statsp.tile([128, 1], F32, name=f"ri{t}", tag="ssq")
                nc.vector.reciprocal(rinv[:], std[:])
                x2 = xnp.tile([128, D], F16, name=f"xn{t}", tag="xn")
                nc.gpsimd.tensor_scalar_mul(x2[:], xt[t][:], rinv[:])
                xn.append(x2)

            # ---------------- transpose xn -> xnT [d, tok] ----------------
            xnT = []
            for dc in range(DC):
                xT = xntp.tile([128, N], F16, name=f"xnT{dc}", tag="xnT")
                for half in range(2):
                    tp = psa.tile([128, 1024], F16, name=f"tp{dc}{half}",
                                  tag="sim")
                    for j in range(8):
                        t = half * 8 + j
                        nc.tensor.transpose(
                            tp[:, j * 128:(j + 1) * 128],
                            xn[t][:, dc * 128:(dc + 1) * 128],
                            identf[:])
                    nc.vector.tensor_copy(
                        xT[:, half * 1024:(half + 1) * 1024], tp[:])
                xnT.append(xT)

            # ---------------- projections ----------------
            # kT/qT feature-major pair tiles [128 feats(2 heads), 2048 tok]
            def proj_fmajor(col0, name):
                tiles = []
                for i in range(2):
                    pt = kqp.tile([128, N], F16, name=f"{name}{i}", tag=name,
                                  bufs=2)
                    for half in range(2):
                        ps = psa.tile([128, 1024], F32, name=f"p{name}{i}{half}",
                                      tag="sim")
                        for tc2 in range(2):
                            tcol = half * 1024 + tc2 * 512
                            for dc in range(DC):
                                nc.tensor.matmul(
                                    ps[:, tc2 * 512:(tc2 + 1) * 512],
                                    w_sb[dc][:, col0 + i * 128:col0 + (i + 1) * 128],
                                    xnT[dc][:, tcol:tcol + 512],
                                    start=(dc == 0), stop=(dc == DC - 1))
                        nc.scalar.copy(
                            pt[:, half * 1024:(half + 1) * 1024], ps[:])
                    tiles.append(pt)
                return tiles

            kTp = proj_fmajor(HPC * DH, "kT")
            qTp = proj_fmajor(0, "qT")

            # v token-major, per head [128 k-part, 16 kc * 65] bf16 with a
            # ones column at slot 64 of each kc block (softmax denominator).
            vx = []
            for h in range(HPC):
                v = vxp.tile([128, KC * 65], BF16, name=f"vx{h}", tag="vx")
                nc.any.memset(
                    v[:].rearrange("p (kc c) -> p kc c", c=65)[:, :, 64:65],
                    1.0)
                vx.append(v)
            for t in range(NT):
                ps = psa.tile([128, 1024], F32, name=f"pv{t}", tag="sim")
                for dc in range(DC):
                    nc.tensor.matmul(
                        ps[:, 0:HPC * DH],
                        xnT[dc][:, t * 128:(t + 1) * 128],
                        w_sb[dc][:, 2 * HPC * DH:3 * HPC * DH],
                        start=(dc == 0), stop=(dc == DC - 1))
                for h in range(HPC):
                    nc.scalar.copy(
                        vx[h][:, t * 65:t * 65 + 64],
                        ps[:, h * 64:(h + 1) * 64])

            # ---------------- attention (software-pipelined) ----------------
            # Unit = (head, 128-query tile). front() runs sim -> max -> exp ->
            # DMA transpose; back() runs AV + normalize. back(u) is emitted
            # LAG units after front(u) so the in-order PE never stalls on the
            # cross-engine max/exp/transpose chain.
            aout_tiles = {}  # (hp, qt) -> [128 q, 128 f] bf16 pair tile
            aoutT = {0: None, 1: None}
            unit_state = {}

            def front(h, qt):
                i, row = h // 2, (h % 2) * 64
                sims = []
                sa = statsp.tile([128, 4], F32, name=f"sa{h}{qt}", tag="sa")
                for half in range(2):
                    ps = psa.tile([128, 1024], F32, name=f"s{h}{qt}{half}",
                                  tag="sim")
                    for kc2 in range(2):
                        kcol = half * 1024 + kc2 * 512
                        nc.tensor.matmul(
                            ps[:, kc2 * 512:(kc2 + 1) * 512],
                            qTp[i][row:row + 64, qt * 128:(qt + 1) * 128],
                            kTp[i][row:row + 64, kcol:kcol + 512],
                            start=True, stop=True)
                    # quarter maxes of this half in one 3D-AP reduce
                    nc.vector.tensor_reduce(
                        sa[:, 2 * half:2 * half + 2],
                        ps[:].rearrange("p (a b) -> p a b", a=2),
                        axis=mybir.AxisListType.X, op=ALU.max)
                    sims.append(ps)
                negm = statsp.tile([128, 1], F32, name=f"nm{h}{qt}", tag="nm")
                nc.vector.tensor_reduce(negm[:], sa[:],
                                        axis=mybir.AxisListType.X,
                                        op=ALU.max, negate=True)
                at = attnp.tile([128, N], BF16, name=f"at{h}{qt}", tag="at")
                for half in range(2):
                    nc.scalar.activation(
                        at[:, half * 1024:(half + 1) * 1024],
                        sims[half][:], AF.Exp, bias=negm[:])
                atT = attntp.tile([128, KC * 128], BF16, name=f"atT{h}{qt}",
                                  tag="atT")
                nc.sync.dma_start_transpose(
                    atT[:].rearrange("p (kc q) -> p kc q", q=128), at[:])
                unit_state[(h, qt)] = atT

            def back(h, qt):
                atT = unit_state.pop((h, qt))
                av = psb.tile([128, 65], F32, name=f"av{h}{qt}", tag="av")
                atT3 = atT[:].rearrange("p (kc q) -> p kc q", q=128)
                for kc in range(KC):
                    nc.tensor.matmul(
                        av[:],
                        atT3[:, kc, :],
                        vx[h][:, kc * 65:(kc + 1) * 65],
                        start=(kc == 0), stop=(kc == KC - 1))
                rs = statsp.tile([128, 1], F32, name=f"rs{h}{qt}", tag="rs")
                nc.vector.reciprocal(rs[:], av[:, 64:65])
                hp = h // 2
                if (hp, qt) not in aout_tiles:
                    aout_tiles[(hp, qt)] = aoutp.tile(
                        [128, 128], BF16, name=f"ao{hp}{qt}", tag="ao")
                nc.scalar.activation(
                    aout_tiles[(hp, qt)][:, (h % 2) * 64:(h % 2) * 64 + 64],
                    av[:, 0:64], AF.Copy, scale=rs[:])

            def aout_transpose(hp, half):
                # transpose this half's aout pair tiles into aoutT[hp]
                if aoutT[hp] is None:
                    aoutT[hp] = aouttp.tile([128, N], BF16, name=f"aoT{hp}",
                                            tag="aT")
                aT = aoutT[hp]
                tp = psa.tile([128, 1024], BF16, name=f"tpa{hp}{half}",
                              tag="sim")
                for j in range(8):
                    qt = half * 8 + j
                    nc.tensor.transpose(
                        tp[:, j * 128:(j + 1) * 128],
                        aout_tiles[(hp, qt)][:], identb[:])
                nc.vector.tensor_copy(
                    aT[:, half * 1024:(half + 1) * 1024], tp[:])

            def outproj(qt):
                ps = psa.tile([128, 1024], F32, name=f"po{qt}", tag="sim")
                for oc in range(2):
                    for hp in range(2):
                        nc.tensor.matmul(
                            ps[:, oc * 512:(oc + 1) * 512],
                            aoutT[hp][:, qt * 128:(qt + 1) * 128],
                            wout_sb[hp][:, oc * 512:(oc + 1) * 512],
                            start=(hp == 0), stop=(hp == 1))
                ot = osbp.tile([128, D], BF16, name=f"o{qt}", tag="o")
                nc.scalar.copy(ot[:], ps[:])
                nc.sync.dma_start(outp_d[qt * 128:(qt + 1) * 128, :], ot[:])

            def reduce_scatter(half):
                import os as _os
                if _os.environ.get("KERNEL_FAKE_COMM") == "1":
                    nc.sync.dma_start(
                        rsout_d[half * 256:(half + 1) * 256, :],
                        outp_d[half * 1024:half * 1024 + 256, :])
                else:
                    nc.gpsimd.collective_compute(
                        "ReduceScatter", ALU.add, replica_groups=rg,
                        ins=[outp_d[half * 1024:(half + 1) * 1024, :].opt()],
                        outs=[rsout_d[half * 256:(half + 1) * 256, :].opt()])
                nc.sync.dma_start(out_d[half * 256:(half + 1) * 256, :],
                                  rsout_d[half * 256:(half + 1) * 256, :])

            # token-half outer loop so half A's outproj + reduce-scatter run
            # under half B's attention
            LAG = 4
            units = [(h, half * 8 + j)
                     for half in range(2) for h in range(HPC) for j in range(8)]
            backlog = []
            for idx, (h, qt) in enumerate(units):
                front(h, qt)
                backlog.append((h, qt))
                if len(backlog) > LAG:
                    bh, bqt = backlog.pop(0)
                    back(bh, bqt)
                    # after finishing both heads of a pair for a half,
                    # transpose; after both pairs, outproj + RS for the half
                    if bh % 2 == 1 and bqt % 8 == 7:
                        aout_transpose(bh // 2, bqt // 8)
                    if bh == HPC - 1 and bqt % 8 == 7:
                        half = bqt // 8
                        for j in range(8):
                            outproj(half * 8 + j)
                        reduce_scatter(half)
            while backlog:
                bh, bqt = backlog.pop(0)
                back(bh, bqt)
                if bh % 2 == 1 and bqt % 8 == 7:
                    aout_transpose(bh // 2, bqt // 8)
                if bh == HPC - 1 and bqt % 8 == 7:
                    half = bqt // 8
                    for j in range(8):
                        outproj(half * 8 + j)
                    reduce_scatter(half)

    nc.finalize()
    return nc


_NC_CACHE = None


def kernel(x, mask, gamma, w_qkv, w_out):
    global _NC_CACHE
    x = np.asarray(x, dtype=np.float32)
    gamma = np.asarray(gamma, dtype=np.float32)
    w_qkv = np.asarray(w_qkv, dtype=np.float32)
    w_out = np.asarray(w_out, dtype=np.float32)

    # fold gamma (RMSNorm scale) and the x8 q-scale into w_qkv (exact in f32)
    w = w_qkv * gamma[:, None]
    w = np.concatenate([w[:, :D] * (DH ** 0.5), w[:, D:]], axis=1)

    if _NC_CACHE is None:
        _NC_CACHE = build_graph()
    nc = _NC_CACHE

    in_maps = []
    for c in range(NC_TOTAL):
        b, hg = divmod(c, GROUP)
        cs = slice(hg * HPC * DH, (hg + 1) * HPC * DH)
        wq = w[:, 0:D][:, cs]
        wk = w[:, D:2 * D][:, cs]
        wv = w[:, 2 * D:3 * D][:, cs]
        wc = np.ascontiguousarray(
            np.concatenate([wq, wk, wv], axis=1), dtype=np.float16)
        wo = np.ascontiguousarray(
            w_out[cs, :].astype(ml_dtypes.bfloat16))
        xs = np.ascontiguousarray(x[b], dtype=np.float16)
        in_maps.append({"x": xs, "w_qkv": wc, "w_out": wo})

    res = run_bass_kernel_spmd(nc, in_maps, core_ids=list(range(NC_TOTAL)))
    out = np.empty((B, N, D), dtype=np.float32)
    for c in range(NC_TOTAL):
        b, r = divmod(c, GROUP)
        o = np.asarray(res.results[c]["out"]).astype(np.float32)
        out[b, r * 256:(r + 1) * 256, :] = o[0:256]
        out[b, N // 2 + r * 256:N // 2 + (r + 1) * 256, :] = o[256:512]
    return out
